# revision 1
# baseline (speedup 1.0000x reference)
"""Trainium2 Bass kernel for nn_AttentionAggregate_Weight (gnn_message_passing).

Computes, per node n with K=32 neighbors and D=128 features:
    score[n,k] = tanh(nodes_key[n].v1 + middle_key[n,k].v2 + a_b)
    out[n,:]   = sum_k softmax_k(score)[n,k] * middle_value[n,k,:]
where v1 = W1.T @ a_w and v2 = W2.T @ a_w are folded on the host (the
reference's p1/p2 projections only ever appear dotted with a_w; tanh
outputs lie in [-1,1] so the softmax needs no max subtraction and the
denominator folds into one final per-node scale).

Distribution: pure data parallel over the node axis across 8 NeuronCores
(2500 nodes each); the tiny folded params are replicated. On-core layout
puts 125 nodes on partitions and (k, d) on the free axis. All compute is
per-k fused DVE ops (scalar_tensor_tensor multiply+row-sum for scores,
multiply+accumulate chains for the weighted values) — small free dims
dodge the DVE's per-op pipeline-DRAIN penalty — with ACT doing tanh and
a fused exp+row-sum. The tile loop is software-pipelined two tiles deep
(scores of tile t emitted before values of tile t-2) so the in-order
engines never stall on the cross-engine score->coefficient chain, and
the big key/value loads stream on the HWDGE ring at the HBM roofline.

Self-contained: hardcodes shapes/sharding; no file I/O.
"""

from contextlib import ExitStack

import numpy as np

N, K, D = 20000, 32, 128
KD = K * D
NCONST = KD + D + 1
N_CORES = 8
NPC = N // N_CORES
P = 125  # nodes per tile (partition dim)
SKEW = 2
BUFS = 3


# ---------------------------------------------------------------------------
# Wait legalization: this walrus build accepts at most ONE semaphore wait per
# instruction; split extras onto same-engine Drain carriers at the BIR level.
# ---------------------------------------------------------------------------
def _legalize_bir_waits(bir_bytes: bytes) -> bytes:
    import orjson

    m = orjson.loads(bir_bytes)
    n = 0
    for f in m.get("functions", []):
        for b in f.get("blocks", []):
            insts = b.get("instructions") or []
            out = []
            changed = False
            for ins in insts:
                si = ins.get("sync_info")
                waits = (si or {}).get("on_wait") or []
                if len(waits) > 1:
                    changed = True
                    for w in waits[:-1]:
                        n += 1
                        out.append(
                            {
                                "debug": ins.get("debug", 0),
                                "engine": ins.get("engine"),
                                "ins": [],
                                "outs": [],
                                "name": f"I-wfix-{n}",
                                "opcode": "Drain",
                                "sync_info": {"on_update": [], "on_wait": [w]},
                            }
                        )
                    si["on_wait"] = [waits[-1]]
                out.append(ins)
            if changed:
                b["instructions"] = out
    return orjson.dumps(m)


_waitfix_installed = False


def _install_waitfix():
    global _waitfix_installed
    if _waitfix_installed:
        return
    import concourse.bass as bass

    orig = bass.Bass.to_json_bytes

    def patched(self):
        return _legalize_bir_waits(orig(self))

    bass.Bass.to_json_bytes = patched
    _waitfix_installed = True


# ---------------------------------------------------------------------------
# Kernel builder (per-core: NPC nodes)
# ---------------------------------------------------------------------------
def _build_kernel():
    import concourse.bass as bass
    import concourse.tile as tile
    from concourse import mybir

    f32 = mybir.dt.float32
    n_tiles = NPC // P

    nc = bass.Bass()
    mk = nc.dram_tensor("mk", (NPC, K, D), f32, kind="ExternalInput")
    nk = nc.dram_tensor("nk", (NPC, D), f32, kind="ExternalInput")
    mv = nc.dram_tensor("mv", (NPC, K, D), f32, kind="ExternalInput")
    consts = nc.dram_tensor("consts", (128, NCONST), f32, kind="ExternalInput")
    out = nc.dram_tensor("out", (NPC, D), f32, kind="ExternalOutput")

    with tile.TileContext(nc) as tc, ExitStack() as ctx:
        singles = ctx.enter_context(tc.tile_pool(name="singles", bufs=1))
        keys = ctx.enter_context(tc.tile_pool(name="keys", bufs=BUFS + SKEW))
        vals = ctx.enter_context(tc.tile_pool(name="vals", bufs=BUFS + SKEW))
        nks = ctx.enter_context(tc.tile_pool(name="nks", bufs=BUFS))
        outs = ctx.enter_context(tc.tile_pool(name="outs", bufs=BUFS))
        smalls = ctx.enter_context(tc.tile_pool(name="smalls", bufs=BUFS + SKEW))
        junks = ctx.enter_context(tc.tile_pool(name="junks", bufs=2))

        ct = singles.tile([128, NCONST], f32)
        nc.gpsimd.dma_start(out=ct, in_=consts[:])
        v1_sb = ct[0:P, KD : KD + D]
        ab_sb = ct[0:P, KD + D : KD + D + 1]
        v2row = ct[0:P, 0:D]
        # dummy touch: DVE observes the const-DMA semaphore before the loop
        dum = singles.tile([1, 1], f32)
        nc.vector.tensor_copy(out=dum, in_=ct[0:1, 0:1])

        def emit_loads(t):
            rows = slice(t * P, (t + 1) * P)
            key3 = keys.tile([P, K, D], f32, tag="key3", name=f"key3_{t}")
            nc.sync.dma_start(out=key3, in_=mk[rows])
            val3 = vals.tile([P, K, D], f32, tag="val3", name=f"val3_{t}")
            nc.sync.dma_start(out=val3, in_=mv[rows])
            nk_t = nks.tile([P, D], f32, tag="nk_t", name=f"nk_{t}")
            nc.gpsimd.dma_start(out=nk_t, in_=nk[rows])
            return {"key3": key3, "val3": val3, "nk_t": nk_t}

        def emit_scores(t, h):
            key3, nk_t = h["key3"], h["nk_t"]
            junk = junks.tile([P, D], f32, tag="junk", name=f"junk_{t}")
            s1b = smalls.tile([P, 1], f32, tag="s1b", name=f"s1b_{t}")
            sc_raw = smalls.tile([P, K], f32, tag="sc_raw", name=f"scr_{t}")
            # s1 = a_b + nk.v1 — fused multiply + row-sum
            nc.vector.scalar_tensor_tensor(
                out=junk, in0=nk_t, scalar=1.0, in1=v1_sb,
                op0=mybir.AluOpType.bypass, op1=mybir.AluOpType.mult,
                accum_out=s1b,
            )
            nc.vector.tensor_add(out=s1b, in0=s1b, in1=ab_sb)
            # s2[n,k] = key[n,k].v2 — one fused multiply+row-sum per k
            for k in range(K):
                nc.vector.scalar_tensor_tensor(
                    out=junk, in0=key3[:, k, :], scalar=1.0, in1=v2row,
                    op0=mybir.AluOpType.bypass, op1=mybir.AluOpType.mult,
                    accum_out=sc_raw[:, k : k + 1],
                )
            sc = smalls.tile([P, K], f32, tag="sc", name=f"sc_{t}")
            nc.scalar.activation(
                out=sc, in_=sc_raw, func=mybir.ActivationFunctionType.Tanh,
                bias=s1b, scale=1.0,
            )
            e_t = smalls.tile([P, K], f32, tag="e_t", name=f"e_{t}")
            sums = smalls.tile([P, 1], f32, tag="sums", name=f"sums_{t}")
            nc.scalar.activation(
                out=e_t, in_=sc, func=mybir.ActivationFunctionType.Exp,
                accum_out=sums,
            )
            recip = smalls.tile([P, 1], f32, tag="recip", name=f"recip_{t}")
            nc.vector.reciprocal(out=recip, in_=sums)
            h["e_t"], h["recip"] = e_t, recip

        def emit_values(t, h):
            val3, e_t, recip = h["val3"], h["e_t"], h["recip"]
            rows = slice(t * P, (t + 1) * P)
            out_t = outs.tile([P, D], f32, tag="out_t", name=f"out_{t}")
            # out_t = sum_k val_k * e_k via fused multiply-accumulate chain
            nc.vector.tensor_scalar_mul(
                out=out_t, in0=val3[:, 0, :], scalar1=e_t[:, 0:1]
            )
            for k in range(1, K):
                nc.vector.scalar_tensor_tensor(
                    out=out_t, in0=val3[:, k, :], scalar=e_t[:, k : k + 1],
                    in1=out_t,
                    op0=mybir.AluOpType.mult, op1=mybir.AluOpType.add,
                )
            nc.vector.tensor_scalar_mul(out=out_t, in0=out_t, scalar1=recip)
            nc.gpsimd.dma_start(out=out[rows], in_=out_t)

        handles = {}
        for i in range(n_tiles + SKEW):
            if i < n_tiles:
                h = emit_loads(i)
                emit_scores(i, h)
                handles[i] = h
            j = i - SKEW
            if j >= 0:
                emit_values(j, handles.pop(j))

    return nc


_nc_cache = {}


def _get_nc():
    if "main" not in _nc_cache:
        _install_waitfix()
        nc = _build_kernel()
        nc.finalize()
        _nc_cache["main"] = nc
    return _nc_cache["main"]


def kernel(middle_key, nodes_key, middle_value, W1, W2, a_w, a_b):
    middle_key = np.ascontiguousarray(middle_key, np.float32)
    nodes_key = np.ascontiguousarray(nodes_key, np.float32)
    middle_value = np.ascontiguousarray(middle_value, np.float32)

    v1 = (W1.astype(np.float64).T @ a_w.astype(np.float64)).astype(np.float32)
    v2 = (W2.astype(np.float64).T @ a_w.astype(np.float64)).astype(np.float32)
    row = np.concatenate([np.tile(v2, K), v1, np.float32(a_b[:1])]).astype(np.float32)
    consts = np.ascontiguousarray(np.tile(row[None, :], (128, 1)), np.float32)

    nc = _get_nc()

    in_maps = []
    for c in range(N_CORES):
        s = slice(c * NPC, (c + 1) * NPC)
        in_maps.append(
            {
                "mk": middle_key[s],
                "nk": nodes_key[s],
                "mv": middle_value[s],
                "consts": consts,
            }
        )

    from concourse import bass2jax

    results = bass2jax.run_bass_via_pjrt(nc, in_maps, n_cores=N_CORES)
    return np.concatenate([r["out"] for r in results], axis=0).astype(np.float32)



# revision 3
# speedup vs baseline: 6.8948x; 6.8948x over previous
"""Trainium2 Bass kernel for nn_AttentionAggregate_Weight (gnn_message_passing).

Computes, per node n with K=32 neighbors and D=128 features:
    score[n,k] = tanh(nodes_key[n].v1 + middle_key[n,k].v2 + a_b)
    out[n,:]   = sum_k softmax_k(score)[n,k] * middle_value[n,k,:]
where v1 = W1.T @ a_w and v2 = W2.T @ a_w are folded on the host (the
reference's p1/p2 projections only ever appear dotted with a_w; tanh
outputs lie in [-1,1] so the softmax needs no max subtraction and the
denominator folds into one final per-node scale).

Distribution: pure data parallel over the node axis across 8 NeuronCores.
Nodes are host-padded 20000 -> 20480 so each core gets 2560 = 20 tiles of
exactly 128 nodes; 128-partition DMA destinations run ~2.5x faster than
the 125-partition shapes the node count would naturally give. The big
inputs are staged as bf16 (host cast), halving HBM traffic; rel-err vs
the fp32 reference is ~5e-3, well inside the 2e-2 gate.

Per 128-node tile: middle_key is staged pre-transposed by the host into
tile-major [d, (k, n)] bf16 slabs (the HW xbar transpose path measured
~2.4x slower than plain loads, so the transpose is done once on the
host) and streams in as plain full-rate loads. The K score dot-products
then run on TensorE as 32 tiny matmuls (stationary = contiguous
mkT[:, k, :] slice, moving = v2 column), accumulating straight into a
[128, K] PSUM tile in node-major layout.
ScalarE applies tanh (center-node term enters via the per-partition bias
port) and exp (+fused row-sum); VectorE does the softmax reciprocal and
the weighted value sum as four bf16 multiply-accumulate sub-chains
(2x-packed DVE mode) combined in fp32, and ScalarE applies the final
1/Z scale. The tile loop is software-pipelined two tiles deep so DMA,
PE, ACT and DVE all overlap; outputs return as bf16 and are upcast on
the host.

Self-contained: hardcodes shapes/sharding; no file I/O.
"""

from contextlib import ExitStack

import numpy as np

N, K, D = 20000, 32, 128
N_CORES = 8
NPC = 2560  # padded nodes per core (20 tiles of 128)
NPAD = NPC * N_CORES
P = 128
SKEW = 2
BUFS = 3
NSUB = 4  # value-chain split for fp32 recombine


# ---------------------------------------------------------------------------
# Wait legalization: this walrus build accepts at most ONE semaphore wait per
# instruction; split extras onto same-engine Drain carriers at the BIR level.
# ---------------------------------------------------------------------------
def _legalize_bir_waits(bir_bytes: bytes) -> bytes:
    import orjson

    m = orjson.loads(bir_bytes)
    n = 0
    for f in m.get("functions", []):
        for b in f.get("blocks", []):
            insts = b.get("instructions") or []
            out = []
            changed = False
            for ins in insts:
                si = ins.get("sync_info")
                waits = (si or {}).get("on_wait") or []
                if len(waits) > 1:
                    changed = True
                    for w in waits[:-1]:
                        n += 1
                        out.append(
                            {
                                "debug": ins.get("debug", 0),
                                "engine": ins.get("engine"),
                                "ins": [],
                                "outs": [],
                                "name": f"I-wfix-{n}",
                                "opcode": "Drain",
                                "sync_info": {"on_update": [], "on_wait": [w]},
                            }
                        )
                    si["on_wait"] = [waits[-1]]
                out.append(ins)
            if changed:
                b["instructions"] = out
    return orjson.dumps(m)


_waitfix_installed = False


def _install_waitfix():
    global _waitfix_installed
    if _waitfix_installed:
        return
    import concourse.bass as bass

    orig = bass.Bass.to_json_bytes

    def patched(self):
        return _legalize_bir_waits(orig(self))

    bass.Bass.to_json_bytes = patched
    _waitfix_installed = True


# ---------------------------------------------------------------------------
# Kernel builder (per-core: NPC nodes, P=128 per tile)
# ---------------------------------------------------------------------------
def _build_kernel():
    import concourse.bass as bass
    import concourse.tile as tile
    from concourse import mybir

    f32 = mybir.dt.float32
    bf16 = mybir.dt.bfloat16
    n_tiles = NPC // P
    KSUB = K // NSUB

    nc = bass.Bass()
    # host-pretransposed, tile-major: row block t*D..(t+1)*D is tile t's
    # [d, (k, p)] slab (k-major free so per-k stationary slices are contiguous)
    mk = nc.dram_tensor("mk", (n_tiles * D, P * K), bf16, kind="ExternalInput")
    nk = nc.dram_tensor("nk", (NPC, D), bf16, kind="ExternalInput")
    mv = nc.dram_tensor("mv", (NPC, K, D), bf16, kind="ExternalInput")
    cb_d = nc.dram_tensor("cb", (128, 1 + D), bf16, kind="ExternalInput")
    cf_d = nc.dram_tensor("cf", (128, 1), f32, kind="ExternalInput")
    out = nc.dram_tensor("out", (NPC, D), bf16, kind="ExternalOutput")

    with tile.TileContext(nc) as tc, ExitStack() as ctx:
        singles = ctx.enter_context(tc.tile_pool(name="singles", bufs=1))
        keys = ctx.enter_context(tc.tile_pool(name="keys", bufs=BUFS + SKEW))
        vals = ctx.enter_context(tc.tile_pool(name="vals", bufs=BUFS + SKEW))
        nks = ctx.enter_context(tc.tile_pool(name="nks", bufs=BUFS))
        outs = ctx.enter_context(tc.tile_pool(name="outs", bufs=BUFS))
        smalls = ctx.enter_context(tc.tile_pool(name="smalls", bufs=BUFS + SKEW))
        accs = ctx.enter_context(tc.tile_pool(name="accs", bufs=2))
        psums = ctx.enter_context(tc.tile_pool(name="psums", bufs=2, space="PSUM"))

        cb = singles.tile([128, 1 + D], bf16)
        nc.gpsimd.dma_start(out=cb, in_=cb_d[:])
        cf = singles.tile([128, 1], f32)
        nc.gpsimd.dma_start(out=cf, in_=cf_d[:])
        v2col = cb[:, 0:1]
        v1row = cb[0:P, 1 : 1 + D]
        ab_sb = cf[0:P, 0:1]
        # dummy touches: engines observe the const-DMA semaphores up front
        dum = singles.tile([1, 2], f32)
        nc.vector.tensor_copy(out=dum[:, 0:1], in_=cf[0:1, 0:1])
        nc.scalar.activation(
            out=dum[:, 1:2], in_=cb[0:1, 0:1],
            func=mybir.ActivationFunctionType.Copy,
        )

        def emit_loads(t):
            rows = slice(t * P, (t + 1) * P)
            mkT = keys.tile([D, P * K], bf16, tag="mkT", name=f"mkT_{t}")
            nc.sync.dma_start(out=mkT, in_=mk[t * D : (t + 1) * D])
            val3 = vals.tile([P, K, D], bf16, tag="val3", name=f"val3_{t}")
            nc.scalar.dma_start(out=val3, in_=mv[rows])
            nk_t = nks.tile([P, D], bf16, tag="nk_t", name=f"nk_{t}")
            nc.gpsimd.dma_start(out=nk_t, in_=nk[rows])
            return {"mkT": mkT, "val3": val3, "nk_t": nk_t}

        def emit_scores(t, h):
            nk_t = h["nk_t"]
            junk = smalls.tile([P, D], bf16, tag="junk", name=f"junk_{t}")
            s1b = smalls.tile([P, 1], f32, tag="s1b", name=f"s1b_{t}")
            # s1 = a_b + nk.v1 — fused multiply + row-sum
            nc.vector.scalar_tensor_tensor(
                out=junk, in0=nk_t, scalar=1.0, in1=v1row,
                op0=mybir.AluOpType.bypass, op1=mybir.AluOpType.mult,
                accum_out=s1b,
            )
            nc.vector.tensor_add(out=s1b, in0=s1b, in1=ab_sb)
            # s2[n,k] = key[n,k].v2 on TensorE: stationary = transposed key
            # slice [d, n], moving = v2 [d, 1], PSUM column k
            ps = psums.tile([P, K], f32, tag="ps", name=f"ps_{t}")
            mkT3 = h["mkT"].rearrange("d (k p) -> d k p", p=P)
            for k in range(K):
                nc.tensor.matmul(
                    ps[:, k : k + 1], mkT3[:, k, :], v2col,
                    start=True, stop=True,
                )
            th = smalls.tile([P, K], bf16, tag="th", name=f"th_{t}")
            nc.scalar.activation(
                out=th, in_=ps, func=mybir.ActivationFunctionType.Tanh,
                bias=s1b, scale=1.0,
            )
            e_t = smalls.tile([P, K], f32, tag="e_t", name=f"e_{t}")
            sums = smalls.tile([P, 1], f32, tag="sums", name=f"sums_{t}")
            nc.scalar.activation(
                out=e_t, in_=th, func=mybir.ActivationFunctionType.Exp,
                accum_out=sums,
            )
            recip = smalls.tile([P, 1], f32, tag="recip", name=f"recip_{t}")
            nc.vector.reciprocal(out=recip, in_=sums)
            h["e_t"], h["recip"] = e_t, recip

        def emit_values(t, h):
            val3, e_t, recip = h["val3"], h["e_t"], h["recip"]
            rows = slice(t * P, (t + 1) * P)
            # out_t = sum_k val_k * e_k: bf16 multiply-accumulate sub-chains
            # (DVE 2x packed mode), recombined pairwise ending in fp32
            sub = []
            for c in range(NSUB):
                k0 = c * KSUB
                acc = accs.tile([P, D], bf16, tag=f"acc{c}", name=f"acc{c}_{t}")
                nc.vector.tensor_scalar_mul(
                    out=acc, in0=val3[:, k0, :], scalar1=e_t[:, k0 : k0 + 1]
                )
                for k in range(k0 + 1, k0 + KSUB):
                    nc.vector.scalar_tensor_tensor(
                        out=acc, in0=val3[:, k, :], scalar=e_t[:, k : k + 1],
                        in1=acc,
                        op0=mybir.AluOpType.mult, op1=mybir.AluOpType.add,
                    )
                sub.append(acc)
            while len(sub) > 1:
                nxt = []
                for i in range(0, len(sub) - 1, 2):
                    dst = accs.tile(
                        [P, D], mybir.dt.float32 if len(sub) == 2 else bf16,
                        tag=f"cmb{len(sub)}_{i}", name=f"cmb{len(sub)}_{i}_{t}",
                    )
                    nc.vector.tensor_add(out=dst, in0=sub[i], in1=sub[i + 1])
                    nxt.append(dst)
                if len(sub) % 2:
                    nxt.append(sub[-1])
                sub = nxt
            out_t = outs.tile([P, D], bf16, tag="out_t", name=f"out_{t}")
            nc.scalar.activation(
                out=out_t, in_=sub[0],
                func=mybir.ActivationFunctionType.Copy, scale=recip,
            )
            nc.gpsimd.dma_start(out=out[rows], in_=out_t)

        handles = {}
        for i in range(n_tiles + SKEW):
            if i < n_tiles:
                h = emit_loads(i)
                emit_scores(i, h)
                handles[i] = h
            j = i - SKEW
            if j >= 0:
                emit_values(j, handles.pop(j))

    nc.finalize()
    return nc


_nc_cache = {}


def _get_nc():
    if "main" not in _nc_cache:
        _install_waitfix()
        _nc_cache["main"] = _build_kernel()
    return _nc_cache["main"]


def _host_prep(W1, W2, a_w, a_b):
    import ml_dtypes

    v1 = (W1.astype(np.float64).T @ a_w.astype(np.float64)).astype(np.float32)
    v2 = (W2.astype(np.float64).T @ a_w.astype(np.float64)).astype(np.float32)
    cb = np.zeros((128, 1 + D), np.float32)
    cb[:, 0] = v2
    cb[:, 1:] = v1[None, :]
    cf = np.full((128, 1), np.float32(a_b[0]), np.float32)
    return cb.astype(ml_dtypes.bfloat16), cf


def kernel(middle_key, nodes_key, middle_value, W1, W2, a_w, a_b):
    import ml_dtypes

    bf = ml_dtypes.bfloat16
    # middle_key: pad, then cast+transpose into tile-major [d, (k, p)] slabs
    mk_pad = np.zeros((NPAD, K, D), np.float32)
    mk_pad[:N] = np.ascontiguousarray(middle_key, np.float32)
    mkb = np.ascontiguousarray(
        mk_pad.reshape(NPAD // P, P, K, D).transpose(0, 3, 2, 1).astype(bf)
    ).reshape(-1, P * K)
    nkb = np.zeros((NPAD, D), bf)
    nkb[:N] = np.ascontiguousarray(nodes_key, np.float32).astype(bf)
    mvb = np.zeros((NPAD, K, D), bf)
    mvb[:N] = np.ascontiguousarray(middle_value, np.float32).astype(bf)
    cb, cf = _host_prep(W1, W2, a_w, a_b)

    nc = _get_nc()

    rpc = mkb.shape[0] // N_CORES
    in_maps = []
    for c in range(N_CORES):
        s = slice(c * NPC, (c + 1) * NPC)
        sk = slice(c * rpc, (c + 1) * rpc)
        in_maps.append(
            {"mk": mkb[sk], "nk": nkb[s], "mv": mvb[s], "cb": cb, "cf": cf}
        )

    from concourse import bass2jax

    results = bass2jax.run_bass_via_pjrt(nc, in_maps, n_cores=N_CORES)
    full = np.concatenate([r["out"] for r in results], axis=0)
    return full[:N].astype(np.float32)
